# revision 43
# baseline (speedup 1.0000x reference)
"""GNN message-passing (segment-mean + 3-layer MLP) Trainium2 kernel.

Strategy (8 NeuronCores, SPMD, full inputs in / full output out):
  - Host: assign nodes to 800 blocks of 64 slots (degree-balanced snake) so
    every block's incoming-edge count fits 6 k-tiles of 128 edges.  Blocks
    0-99 -> core 0, etc.  Edges are bucketed per receiver block, pre-scaled
    by 1/deg(recv), cast to bf16, and laid out [eslot, ktile*feat] so
    per-chunk DMAs are large and contiguous.  One-hot scatter masks are 64
    columns wide (fp8), halving mask DMA vs 128-wide blocks.
  - Device per core: segment-mean as mask matmuls on the TensorEngine (6
    k-tiles per block accumulated into one 512-col PSUM bank per chunk),
    then the 3-layer MLP over 512/256-node chunks in feature-major layout.
    Everything except PSUM/bias/mask is bf16: halves DMA bytes and enables
    fast-weight-load on the PE (f32r disables FWL).  ~3us of dummy matmuls
    at program start ramp the PE p-state while the DMA pipeline fills.
    Edge/mask slabs stream on the SP HWDGE ring in 2-4 block granules, 4
    chunks deep (the first two chunks are half-sized so compute starts
    early); x/weights/outputs ride the ACT ring, with the three biases
    fused into one transfer and x batched into 4-chunk spans.  Output is
    written bf16 and upcast on the host.
"""
import sys

sys.path.insert(0, "/opt/trn_rl_repo")

import numpy as np
import ml_dtypes

from concourse import bacc
import concourse.mybir as mybir
import concourse.tile as tile
from concourse.bass_utils import run_bass_kernel_spmd

# problem shape (hardcoded per contract)
N_NODES = 50000
N_EDGES = 600000
D = 128          # node/edge feature dim
DH = 512         # hidden dim
C = 8            # cores
W = 64           # node slots per block
BPC = 100        # node blocks per core
NB = C * BPC     # 800 blocks total
SLOTS = BPC * W  # 6400 node slots per core
T_BLK = 6        # edge k-tiles (128 edges) per block
TT = BPC * T_BLK   # k-tiles per core
CHUNKS = [4] * 4 + [8] * 10 + [4]  # blocks per MLP chunk (256-node ramp start)
PREF = 4         # chunks of edge-slab prefetch depth

F32 = mybir.dt.float32
BF16 = mybir.dt.bfloat16
FP8 = mybir.dt.float8e4

_prog_cache = {}
LAST_RESULTS = None  # BassKernelResults of the most recent run (for test.py)


def _build_program(t_blk=T_BLK):
    if t_blk in _prog_cache:
        return _prog_cache[t_blk]
    tt = BPC * t_blk
    nc = bacc.Bacc("TRN2", target_bir_lowering=False)

    xT_d = nc.declare_dram_parameter("xT", [128, SLOTS], BF16, isOutput=False)
    ea_d = nc.declare_dram_parameter("ea", [128, tt * D], BF16, isOutput=False)
    mk_d = nc.declare_dram_parameter("mk", [128, tt * W], FP8, isOutput=False)
    w1_d = nc.declare_dram_parameter("w1", [2 * D, DH], BF16, isOutput=False)
    w2_d = nc.declare_dram_parameter("w2", [DH, DH], BF16, isOutput=False)
    w3_d = nc.declare_dram_parameter("w3", [DH, D], BF16, isOutput=False)
    bb_d = nc.declare_dram_parameter("bb", [128, 9], F32, isOutput=False)
    out_d = nc.declare_dram_parameter("outT", [128, SLOTS], BF16, isOutput=True)

    RELU = mybir.ActivationFunctionType.Relu
    ADD = mybir.AluOpType.add

    n_chunks = len(CHUNKS)
    chunk_blk0 = [0]
    for c_i in range(1, n_chunks):
        chunk_blk0.append(chunk_blk0[-1] + CHUNKS[c_i - 1])

    with tile.TileContext(nc) as tc:
        with (
            tc.tile_pool(name="pers", bufs=1) as pers,
            tc.tile_pool(name="eap", bufs=12) as eap,
            tc.tile_pool(name="mkp", bufs=12) as mkp,
            tc.tile_pool(name="xp", bufs=4) as xp,
            tc.tile_pool(name="actp", bufs=2) as actp,
            tc.tile_pool(name="scat_ps", bufs=3, space="PSUM") as scat_ps,
            tc.tile_pool(name="mlp_ps", bufs=5, space="PSUM") as mlp_ps,
        ):
            # granules[(chunk, block)] = (ea_tile, mk_tile, local_block)
            gran = {}

            def load_granule(c_i, b0, nblk):
                kt0 = (chunk_blk0[c_i] + b0) * t_blk
                nkt = nblk * t_blk
                ea_t = eap.tile([128, 4 * t_blk * D], BF16, tag="ea")
                nc.sync.dma_start(
                    out=ea_t[:, : nkt * D], in_=ea_d[:, kt0 * D : (kt0 + nkt) * D]
                )
                mk_t = mkp.tile([128, 4 * t_blk * W], FP8, tag="mk")
                nc.sync.dma_start(
                    out=mk_t[:, : nkt * W], in_=mk_d[:, kt0 * W : (kt0 + nkt) * W]
                )
                for lb in range(nblk):
                    gran[(c_i, b0 + lb)] = (ea_t, mk_t, lb)

            def load_chunk_slabs(c_i, granule_blks):
                nb = CHUNKS[c_i]
                if isinstance(granule_blks, int):
                    granule_blks = [granule_blks] * ((nb + granule_blks - 1) // granule_blks)
                b0 = 0
                for g in granule_blks:
                    g = min(g, nb - b0)
                    if g <= 0:
                        break
                    load_granule(c_i, b0, g)
                    b0 += g

            x_ts = {}

            def load_x_span(c_lo, c_hi):
                col_lo = chunk_blk0[c_lo] * W
                col_hi = chunk_blk0[c_hi] * W + CHUNKS[c_hi] * W
                xt = xp.tile([128, 2816], BF16, tag="x")
                nc.scalar.dma_start(out=xt[:, : col_hi - col_lo], in_=xT_d[:, col_lo:col_hi])
                for c in range(c_lo, c_hi + 1):
                    x_ts[c] = (xt, chunk_blk0[c] * W - col_lo)

            # --- PE p-state warmup: ~3us of dummy matmuls with no deps run
            # during the DMA pipeline fill, so the real scatter/MLP start at
            # the full 2.4 GHz p-state instead of ramping through it ---
            warm_in = pers.tile([128, 128], BF16)
            nc.vector.memset(warm_in[:], 1.0)
            warm_rhs = pers.tile([128, 512], BF16)
            nc.vector.memset(warm_rhs[:], 1.0)
            warm_ps = scat_ps.tile([128, 512], F32, tag="scat")
            for wi in range(7):
                nc.tensor.matmul(
                    out=warm_ps[:],
                    lhsT=warm_in[:],
                    rhs=warm_rhs[:],
                    start=(wi == 0),
                    stop=(wi == 6),
                )

            # --- edge/mask slabs stream on the SP ring; chunk sizes ramp up
            # so the first scatter/MLP work starts ASAP ---
            load_chunk_slabs(0, 2)
            # --- persistent tiles + x on the ACT ring (w1 first: it gates
            # the first L1; fused biases ride one transfer) ---
            load_x_span(0, 0)
            w1t = pers.tile([128, 2, DH], BF16)
            nc.scalar.dma_start(out=w1t[:], in_=w1_d[:].rearrange("(k p) m -> p k m", p=128))
            bbt = pers.tile([128, 9], F32)
            nc.scalar.dma_start(out=bbt[:], in_=bb_d[:])
            load_chunk_slabs(1, 2)
            w2t = pers.tile([128, 4, DH], BF16)
            nc.scalar.dma_start(out=w2t[:], in_=w2_d[:].rearrange("(k p) m -> p k m", p=128))
            load_chunk_slabs(2, 4)
            w3t = pers.tile([128, 4, D], BF16)
            nc.scalar.dma_start(out=w3t[:], in_=w3_d[:].rearrange("(k p) m -> p k m", p=128))
            load_chunk_slabs(3, 4)
            load_x_span(1, 4)

            def emit_scatter(c_i, b_lo, b_hi, ps):
                for b in range(b_lo, b_hi):
                    ea_t, mk_t, lb = gran.pop((c_i, b))
                    for t in range(t_blk):
                        j = (lb * t_blk + t)
                        nc.tensor.matmul(
                            out=ps[:, b * W : (b + 1) * W],
                            lhsT=ea_t[:, j * D : (j + 1) * D],
                            rhs=mk_t[:, j * W : (j + 1) * W],
                            start=(t == 0),
                            stop=(t == t_blk - 1),
                        )

            col0 = 0
            ps_next = None
            pre_done = 0
            for c_i, nb in enumerate(CHUNKS):
                NCW = nb * W
                # scatter: segment-mean via mask matmuls, one PSUM bank per
                # chunk (a 2-block prefix may already have run, interleaved
                # after the previous chunk's L1)
                if c_i in (2, 3):
                    # keep the PE clocked through the known slab-wait seam:
                    # dependency-free dummies run during the DMA stall so the
                    # p-state does not drop to MID for the following MLP
                    wp = scat_ps.tile([128, 512], F32, tag="scat")
                    for wi in range(6):
                        nc.tensor.matmul(
                            out=wp[:],
                            lhsT=warm_in[:],
                            rhs=warm_rhs[:],
                            start=(wi == 0),
                            stop=(wi == 5),
                        )
                if ps_next is not None:
                    ps = ps_next
                else:
                    ps = scat_ps.tile([128, 512], F32, tag="scat")
                emit_scatter(c_i, pre_done, nb, ps)
                ps_next = None
                pre_done = 0
                mean_t = actp.tile([128, 512], BF16, tag="mean")
                nc.vector.tensor_copy(out=mean_t[:, :NCW], in_=ps[:, :NCW])

                # prefetch a later chunk's slabs / x
                if c_i + PREF < n_chunks:
                    load_chunk_slabs(c_i + PREF, 4)
                if c_i == 0:
                    load_x_span(5, 8)
                elif c_i == 2:
                    load_x_span(9, n_chunks - 1)

                # layer 1: h1 = relu(W1.T @ [x; mean] + b1).  All four
                # x-half matmuls issue first (they depend only on x), hiding
                # the DVE psum->SBUF mean-copy latency behind ~0.9us of PE work.
                h1_t = actp.tile([128, 4, 512], BF16, tag="h1")
                xt_c, xoff = x_ts[c_i]
                pms = []
                for m in range(4):
                    pm = mlp_ps.tile([128, 512], F32, tag="mlp")
                    nc.tensor.matmul(
                        out=pm[:, :NCW],
                        lhsT=w1t[:, 0, m * 128 : (m + 1) * 128],
                        rhs=xt_c[:, xoff : xoff + NCW],
                        start=True,
                        stop=False,
                    )
                    pms.append(pm)
                for m in range(4):
                    nc.tensor.matmul(
                        out=pms[m][:, :NCW],
                        lhsT=w1t[:, 1, m * 128 : (m + 1) * 128],
                        rhs=mean_t[:, :NCW],
                        start=False,
                        stop=True,
                    )
                    if m % 2 == 0:
                        nc.scalar.activation(
                            out=h1_t[:, m, :NCW], in_=pms[m][:, :NCW], func=RELU, bias=bbt[:, m : m + 1]
                        )
                    else:
                        nc.vector.tensor_scalar(
                            out=h1_t[:, m, :NCW], in0=pms[m][:, :NCW], scalar1=bbt[:, m : m + 1],
                            scalar2=0.0, op0=ADD, op1=mybir.AluOpType.max,
                        )
                # next chunk's scatter prefix: independent PE work that
                # bridges the L1-relu -> L2 dependency latency
                if c_i >= 4 and c_i + 1 < n_chunks:
                    ps_next = scat_ps.tile([128, 512], F32, tag="scat")
                    pre_done = min(2, CHUNKS[c_i + 1])
                    emit_scatter(c_i + 1, 0, pre_done, ps_next)

                # layer 2
                h2_t = actp.tile([128, 4, 512], BF16, tag="h2")
                for m in range(4):
                    pm = mlp_ps.tile([128, 512], F32, tag="mlp")
                    for k in range(4):
                        nc.tensor.matmul(
                            out=pm[:, :NCW],
                            lhsT=w2t[:, k, m * 128 : (m + 1) * 128],
                            rhs=h1_t[:, k, :NCW],
                            start=(k == 0),
                            stop=(k == 3),
                        )
                    if m % 2 == 0:
                        nc.scalar.activation(
                            out=h2_t[:, m, :NCW], in_=pm[:, :NCW], func=RELU, bias=bbt[:, 4 + m : 5 + m]
                        )
                    else:
                        nc.vector.tensor_scalar(
                            out=h2_t[:, m, :NCW], in0=pm[:, :NCW], scalar1=bbt[:, 4 + m : 5 + m],
                            scalar2=0.0, op0=ADD, op1=mybir.AluOpType.max,
                        )
                # layer 3: out = W3.T @ h2 + b3  (bias add on DVE, bf16 out)
                pm = mlp_ps.tile([128, 512], F32, tag="mlp")
                for k in range(4):
                    nc.tensor.matmul(
                        out=pm[:, :NCW],
                        lhsT=w3t[:, k, :],
                        rhs=h2_t[:, k, :NCW],
                        start=(k == 0),
                        stop=(k == 3),
                    )
                out_t = actp.tile([128, 512], BF16, tag="out")
                nc.vector.tensor_scalar_add(out_t[:, :NCW], pm[:, :NCW], bbt[:, 8:9])
                nc.scalar.dma_start(out=out_d[:, col0 : col0 + NCW], in_=out_t[:, :NCW])
                col0 += NCW

    nc.compile()
    _prog_cache[t_blk] = nc
    return nc


def _preprocess(x, edge_index, edge_attr):
    recv = np.asarray(edge_index)[1].astype(np.int64)
    deg = np.bincount(recv, minlength=N_NODES)
    # snake assignment of degree-sorted nodes into NB blocks (62-63 nodes each)
    order = np.argsort(-deg, kind="stable")
    i = np.arange(N_NODES)
    rnd, pos = i // NB, i % NB
    blk = np.where(rnd % 2 == 0, pos, NB - 1 - pos)
    node_block = np.empty(N_NODES, np.int64)
    node_slot = np.empty(N_NODES, np.int64)
    node_block[order] = blk
    node_slot[order] = rnd
    node_core = node_block // BPC
    node_col = (node_block % BPC) * W + node_slot

    eb = node_block[recv]
    bc = np.bincount(eb, minlength=NB)
    t_blk = max(T_BLK, int(-(-int(bc.max()) // 128)))  # >= ceil(max_load/128)
    tt = BPC * t_blk

    eorder = np.argsort(eb, kind="stable")
    eb_s = eb[eorder]
    starts = np.zeros(NB, np.int64)
    starts[1:] = np.cumsum(bc)[:-1]
    ewithin = np.arange(N_EDGES) - starts[eb_s]
    ktile = ewithin // 128
    eslot = ewithin % 128
    ecore = eb_s // BPC
    kt_in_core = (eb_s % BPC) * t_blk + ktile

    # scale edges by 1/deg(recv) on the host, then cast once to bf16
    ea_scaled = np.asarray(edge_attr, np.float32) * (1.0 / deg[recv])[:, None].astype(np.float32)
    ea_bf = ea_scaled.astype(ml_dtypes.bfloat16)
    ea_buf = np.zeros((C, tt, 128, D), ml_dtypes.bfloat16)
    ea_buf[ecore, kt_in_core, eslot] = ea_bf[eorder]
    # 0/1 scatter masks in fp8, 64 columns per block
    mk_buf = np.zeros((C, tt, 128, W), ml_dtypes.float8_e4m3)
    mk_buf[ecore, kt_in_core, eslot, (node_col[recv] % W)[eorder]] = 1.0

    X_all = np.zeros((C, SLOTS, D), ml_dtypes.bfloat16)
    X_all[node_core, node_col] = np.asarray(x, np.float32).astype(ml_dtypes.bfloat16)

    shards = []
    for c in range(C):
        shards.append(
            dict(
                xT=np.ascontiguousarray(X_all[c].T),
                ea=np.ascontiguousarray(ea_buf[c].transpose(1, 0, 2).reshape(128, tt * D)),
                mk=np.ascontiguousarray(mk_buf[c].transpose(1, 0, 2).reshape(128, tt * W)),
            )
        )
    return shards, node_core, node_col, t_blk


def kernel(x, edge_index, edge_attr, W1, b1, W2, b2, W3, b3, _trace=False):
    global LAST_RESULTS
    shards, node_core, node_col, t_blk = _preprocess(x, edge_index, edge_attr)

    W1 = np.ascontiguousarray(np.asarray(W1, np.float32).astype(ml_dtypes.bfloat16))
    W2 = np.ascontiguousarray(np.asarray(W2, np.float32).astype(ml_dtypes.bfloat16))
    W3 = np.ascontiguousarray(np.asarray(W3, np.float32).astype(ml_dtypes.bfloat16))
    bb = np.concatenate(
        [
            np.asarray(b1, np.float32).reshape(4, 128).T,
            np.asarray(b2, np.float32).reshape(4, 128).T,
            np.asarray(b3, np.float32).reshape(1, 128).T,
        ],
        axis=1,
    )
    bb = np.ascontiguousarray(bb)

    in_maps = []
    for c in range(C):
        m = dict(shards[c])
        m.update(w1=W1, w2=W2, w3=W3, bb=bb)
        in_maps.append(m)

    nc = _build_program(t_blk)
    res = run_bass_kernel_spmd(nc, in_maps, core_ids=list(range(C)), trace=_trace)
    LAST_RESULTS = res

    outs = np.stack([res.results[c]["outT"] for c in range(C)])  # [C, 128, SLOTS] bf16
    out = outs.transpose(0, 2, 1)[node_core, node_col]
    return np.ascontiguousarray(out, dtype=np.float32)


# revision 44
# speedup vs baseline: 1.0003x; 1.0003x over previous
"""GNN message-passing (segment-mean + 3-layer MLP) Trainium2 kernel.

Strategy (8 NeuronCores, SPMD, full inputs in / full output out):
  - Host: assign nodes to 800 blocks of 64 slots (degree-balanced snake) so
    every block's incoming-edge count fits 6 k-tiles of 128 edges.  Blocks
    0-99 -> core 0, etc.  Edges are bucketed per receiver block, pre-scaled
    by 1/deg(recv), cast to bf16, and laid out [eslot, ktile*feat] so
    per-chunk DMAs are large and contiguous.  One-hot scatter masks are 64
    columns wide (fp8), halving mask DMA vs 128-wide blocks.
  - Device per core: segment-mean as mask matmuls on the TensorEngine (6
    k-tiles per block accumulated into one 512-col PSUM bank per chunk),
    then the 3-layer MLP over 512/256-node chunks in feature-major layout.
    Everything except PSUM/bias/mask is bf16: halves DMA bytes and enables
    fast-weight-load on the PE (f32r disables FWL).  ~3us of dummy matmuls
    at program start ramp the PE p-state while the DMA pipeline fills.
    Edge/mask slabs stream on the SP HWDGE ring in 2-4 block granules, 4
    chunks deep (the first two chunks are half-sized so compute starts
    early); x/weights/outputs ride the ACT ring, with the three biases
    fused into one transfer and x batched into 4-chunk spans.  Output is
    written bf16 and upcast on the host.
"""
import sys

sys.path.insert(0, "/opt/trn_rl_repo")

import numpy as np
import ml_dtypes

from concourse import bacc
import concourse.mybir as mybir
import concourse.tile as tile
from concourse.bass_utils import run_bass_kernel_spmd

# problem shape (hardcoded per contract)
N_NODES = 50000
N_EDGES = 600000
D = 128          # node/edge feature dim
DH = 512         # hidden dim
C = 8            # cores
W = 64           # node slots per block
BPC = 100        # node blocks per core
NB = C * BPC     # 800 blocks total
SLOTS = BPC * W  # 6400 node slots per core
T_BLK = 6        # edge k-tiles (128 edges) per block
TT = BPC * T_BLK   # k-tiles per core
CHUNKS = [4] * 4 + [8] * 10 + [4]  # blocks per MLP chunk (256-node ramp start)
PREF = 4         # chunks of edge-slab prefetch depth

F32 = mybir.dt.float32
BF16 = mybir.dt.bfloat16
FP8 = mybir.dt.float8e4

_prog_cache = {}
LAST_RESULTS = None  # BassKernelResults of the most recent run (for test.py)


def _build_program(t_blk=T_BLK):
    if t_blk in _prog_cache:
        return _prog_cache[t_blk]
    tt = BPC * t_blk
    nc = bacc.Bacc("TRN2", target_bir_lowering=False)

    xT_d = nc.declare_dram_parameter("xT", [128, SLOTS], BF16, isOutput=False)
    ea_d = nc.declare_dram_parameter("ea", [128, tt * D], BF16, isOutput=False)
    mk_d = nc.declare_dram_parameter("mk", [128, tt * W], FP8, isOutput=False)
    w1_d = nc.declare_dram_parameter("w1", [2 * D, DH], BF16, isOutput=False)
    w2_d = nc.declare_dram_parameter("w2", [DH, DH], BF16, isOutput=False)
    w3_d = nc.declare_dram_parameter("w3", [DH, D], BF16, isOutput=False)
    bb_d = nc.declare_dram_parameter("bb", [128, 9], F32, isOutput=False)
    out_d = nc.declare_dram_parameter("outT", [128, SLOTS], BF16, isOutput=True)

    RELU = mybir.ActivationFunctionType.Relu
    ADD = mybir.AluOpType.add

    n_chunks = len(CHUNKS)
    chunk_blk0 = [0]
    for c_i in range(1, n_chunks):
        chunk_blk0.append(chunk_blk0[-1] + CHUNKS[c_i - 1])

    with tile.TileContext(nc) as tc:
        with (
            tc.tile_pool(name="pers", bufs=1) as pers,
            tc.tile_pool(name="eap", bufs=12) as eap,
            tc.tile_pool(name="mkp", bufs=12) as mkp,
            tc.tile_pool(name="xp", bufs=4) as xp,
            tc.tile_pool(name="actp", bufs=2) as actp,
            tc.tile_pool(name="scat_ps", bufs=3, space="PSUM") as scat_ps,
            tc.tile_pool(name="mlp_ps", bufs=5, space="PSUM") as mlp_ps,
        ):
            # granules[(chunk, block)] = (ea_tile, mk_tile, local_block)
            gran = {}

            def load_granule(c_i, b0, nblk):
                kt0 = (chunk_blk0[c_i] + b0) * t_blk
                nkt = nblk * t_blk
                ea_t = eap.tile([128, 4 * t_blk * D], BF16, tag="ea")
                nc.sync.dma_start(
                    out=ea_t[:, : nkt * D], in_=ea_d[:, kt0 * D : (kt0 + nkt) * D]
                )
                mk_t = mkp.tile([128, 4 * t_blk * W], FP8, tag="mk")
                nc.sync.dma_start(
                    out=mk_t[:, : nkt * W], in_=mk_d[:, kt0 * W : (kt0 + nkt) * W]
                )
                for lb in range(nblk):
                    gran[(c_i, b0 + lb)] = (ea_t, mk_t, lb)

            def load_chunk_slabs(c_i, granule_blks):
                nb = CHUNKS[c_i]
                if isinstance(granule_blks, int):
                    granule_blks = [granule_blks] * ((nb + granule_blks - 1) // granule_blks)
                b0 = 0
                for g in granule_blks:
                    g = min(g, nb - b0)
                    if g <= 0:
                        break
                    load_granule(c_i, b0, g)
                    b0 += g

            x_ts = {}

            def load_x_span(c_lo, c_hi):
                col_lo = chunk_blk0[c_lo] * W
                col_hi = chunk_blk0[c_hi] * W + CHUNKS[c_hi] * W
                xt = xp.tile([128, 2816], BF16, tag="x")
                nc.scalar.dma_start(out=xt[:, : col_hi - col_lo], in_=xT_d[:, col_lo:col_hi])
                for c in range(c_lo, c_hi + 1):
                    x_ts[c] = (xt, chunk_blk0[c] * W - col_lo)

            # --- PE p-state warmup: ~3us of dummy matmuls with no deps run
            # during the DMA pipeline fill, so the real scatter/MLP start at
            # the full 2.4 GHz p-state instead of ramping through it ---
            warm_in = pers.tile([128, 128], BF16)
            nc.vector.memset(warm_in[:], 1.0)
            warm_rhs = pers.tile([128, 512], BF16)
            nc.vector.memset(warm_rhs[:], 1.0)
            warm_ps = scat_ps.tile([128, 512], F32, tag="scat")
            for wi in range(7):
                nc.tensor.matmul(
                    out=warm_ps[:],
                    lhsT=warm_in[:],
                    rhs=warm_rhs[:],
                    start=(wi == 0),
                    stop=(wi == 6),
                )

            # --- edge/mask slabs stream on the SP ring; chunk sizes ramp up
            # so the first scatter/MLP work starts ASAP ---
            load_chunk_slabs(0, 2)
            # --- persistent tiles + x on the ACT ring (w1 first: it gates
            # the first L1; fused biases ride one transfer) ---
            load_x_span(0, 0)
            w1t = pers.tile([128, 2, DH], BF16)
            nc.scalar.dma_start(out=w1t[:], in_=w1_d[:].rearrange("(k p) m -> p k m", p=128))
            bbt = pers.tile([128, 9], F32)
            nc.scalar.dma_start(out=bbt[:], in_=bb_d[:])
            load_chunk_slabs(1, 2)
            w2t = pers.tile([128, 4, DH], BF16)
            nc.scalar.dma_start(out=w2t[:], in_=w2_d[:].rearrange("(k p) m -> p k m", p=128))
            load_chunk_slabs(2, 4)
            w3t = pers.tile([128, 4, D], BF16)
            nc.scalar.dma_start(out=w3t[:], in_=w3_d[:].rearrange("(k p) m -> p k m", p=128))
            load_chunk_slabs(3, 4)
            load_x_span(1, 4)

            def emit_scatter(c_i, b_lo, b_hi, ps):
                for b in range(b_lo, b_hi):
                    ea_t, mk_t, lb = gran.pop((c_i, b))
                    for t in range(t_blk):
                        j = (lb * t_blk + t)
                        nc.tensor.matmul(
                            out=ps[:, b * W : (b + 1) * W],
                            lhsT=ea_t[:, j * D : (j + 1) * D],
                            rhs=mk_t[:, j * W : (j + 1) * W],
                            start=(t == 0),
                            stop=(t == t_blk - 1),
                        )

            col0 = 0
            ps_next = None
            pre_done = 0
            for c_i, nb in enumerate(CHUNKS):
                NCW = nb * W
                # scatter: segment-mean via mask matmuls, one PSUM bank per
                # chunk (a 2-block prefix may already have run, interleaved
                # after the previous chunk's L1)
                if ps_next is not None:
                    ps = ps_next
                else:
                    ps = scat_ps.tile([128, 512], F32, tag="scat")
                emit_scatter(c_i, pre_done, nb, ps)
                ps_next = None
                pre_done = 0
                mean_t = actp.tile([128, 512], BF16, tag="mean")
                nc.vector.tensor_copy(out=mean_t[:, :NCW], in_=ps[:, :NCW])

                # prefetch a later chunk's slabs / x
                if c_i + PREF < n_chunks:
                    load_chunk_slabs(c_i + PREF, 4)
                if c_i == 0:
                    load_x_span(5, 8)
                elif c_i == 2:
                    load_x_span(9, n_chunks - 1)

                # layer 1: h1 = relu(W1.T @ [x; mean] + b1).  All four
                # x-half matmuls issue first (they depend only on x), hiding
                # the DVE psum->SBUF mean-copy latency behind ~0.9us of PE work.
                h1_t = actp.tile([128, 4, 512], BF16, tag="h1")
                xt_c, xoff = x_ts[c_i]
                pms = []
                for m in range(4):
                    pm = mlp_ps.tile([128, 512], F32, tag="mlp")
                    nc.tensor.matmul(
                        out=pm[:, :NCW],
                        lhsT=w1t[:, 0, m * 128 : (m + 1) * 128],
                        rhs=xt_c[:, xoff : xoff + NCW],
                        start=True,
                        stop=False,
                    )
                    pms.append(pm)
                for m in range(4):
                    nc.tensor.matmul(
                        out=pms[m][:, :NCW],
                        lhsT=w1t[:, 1, m * 128 : (m + 1) * 128],
                        rhs=mean_t[:, :NCW],
                        start=False,
                        stop=True,
                    )
                    if m % 2 == 0:
                        nc.scalar.activation(
                            out=h1_t[:, m, :NCW], in_=pms[m][:, :NCW], func=RELU, bias=bbt[:, m : m + 1]
                        )
                    else:
                        nc.vector.tensor_scalar(
                            out=h1_t[:, m, :NCW], in0=pms[m][:, :NCW], scalar1=bbt[:, m : m + 1],
                            scalar2=0.0, op0=ADD, op1=mybir.AluOpType.max,
                        )
                # next chunk's scatter prefix: independent PE work that
                # bridges the L1-relu -> L2 dependency latency
                if c_i >= 4 and c_i + 1 < n_chunks:
                    ps_next = scat_ps.tile([128, 512], F32, tag="scat")
                    pre_done = min(2, CHUNKS[c_i + 1])
                    emit_scatter(c_i + 1, 0, pre_done, ps_next)

                # layer 2
                h2_t = actp.tile([128, 4, 512], BF16, tag="h2")
                for m in range(4):
                    pm = mlp_ps.tile([128, 512], F32, tag="mlp")
                    for k in range(4):
                        nc.tensor.matmul(
                            out=pm[:, :NCW],
                            lhsT=w2t[:, k, m * 128 : (m + 1) * 128],
                            rhs=h1_t[:, k, :NCW],
                            start=(k == 0),
                            stop=(k == 3),
                        )
                    if m % 2 == 0:
                        nc.scalar.activation(
                            out=h2_t[:, m, :NCW], in_=pm[:, :NCW], func=RELU, bias=bbt[:, 4 + m : 5 + m]
                        )
                    else:
                        nc.vector.tensor_scalar(
                            out=h2_t[:, m, :NCW], in0=pm[:, :NCW], scalar1=bbt[:, 4 + m : 5 + m],
                            scalar2=0.0, op0=ADD, op1=mybir.AluOpType.max,
                        )
                # layer 3: out = W3.T @ h2 + b3  (bias add on DVE, bf16 out)
                pm = mlp_ps.tile([128, 512], F32, tag="mlp")
                for k in range(4):
                    nc.tensor.matmul(
                        out=pm[:, :NCW],
                        lhsT=w3t[:, k, :],
                        rhs=h2_t[:, k, :NCW],
                        start=(k == 0),
                        stop=(k == 3),
                    )
                out_t = actp.tile([128, 512], BF16, tag="out")
                nc.vector.tensor_scalar_add(out_t[:, :NCW], pm[:, :NCW], bbt[:, 8:9])
                nc.scalar.dma_start(out=out_d[:, col0 : col0 + NCW], in_=out_t[:, :NCW])
                col0 += NCW

    nc.compile()
    _prog_cache[t_blk] = nc
    return nc


def _preprocess(x, edge_index, edge_attr):
    recv = np.asarray(edge_index)[1].astype(np.int64)
    deg = np.bincount(recv, minlength=N_NODES)
    # snake assignment of degree-sorted nodes into NB blocks (62-63 nodes each)
    order = np.argsort(-deg, kind="stable")
    i = np.arange(N_NODES)
    rnd, pos = i // NB, i % NB
    blk = np.where(rnd % 2 == 0, pos, NB - 1 - pos)
    node_block = np.empty(N_NODES, np.int64)
    node_slot = np.empty(N_NODES, np.int64)
    node_block[order] = blk
    node_slot[order] = rnd
    node_core = node_block // BPC
    node_col = (node_block % BPC) * W + node_slot

    eb = node_block[recv]
    bc = np.bincount(eb, minlength=NB)
    t_blk = max(T_BLK, int(-(-int(bc.max()) // 128)))  # >= ceil(max_load/128)
    tt = BPC * t_blk

    eorder = np.argsort(eb, kind="stable")
    eb_s = eb[eorder]
    starts = np.zeros(NB, np.int64)
    starts[1:] = np.cumsum(bc)[:-1]
    ewithin = np.arange(N_EDGES) - starts[eb_s]
    ktile = ewithin // 128
    eslot = ewithin % 128
    ecore = eb_s // BPC
    kt_in_core = (eb_s % BPC) * t_blk + ktile

    # scale edges by 1/deg(recv) on the host, then cast once to bf16
    ea_scaled = np.asarray(edge_attr, np.float32) * (1.0 / deg[recv])[:, None].astype(np.float32)
    ea_bf = ea_scaled.astype(ml_dtypes.bfloat16)
    ea_buf = np.zeros((C, tt, 128, D), ml_dtypes.bfloat16)
    ea_buf[ecore, kt_in_core, eslot] = ea_bf[eorder]
    # 0/1 scatter masks in fp8, 64 columns per block
    mk_buf = np.zeros((C, tt, 128, W), ml_dtypes.float8_e4m3)
    mk_buf[ecore, kt_in_core, eslot, (node_col[recv] % W)[eorder]] = 1.0

    X_all = np.zeros((C, SLOTS, D), ml_dtypes.bfloat16)
    X_all[node_core, node_col] = np.asarray(x, np.float32).astype(ml_dtypes.bfloat16)

    shards = []
    for c in range(C):
        shards.append(
            dict(
                xT=np.ascontiguousarray(X_all[c].T),
                ea=np.ascontiguousarray(ea_buf[c].transpose(1, 0, 2).reshape(128, tt * D)),
                mk=np.ascontiguousarray(mk_buf[c].transpose(1, 0, 2).reshape(128, tt * W)),
            )
        )
    return shards, node_core, node_col, t_blk


def kernel(x, edge_index, edge_attr, W1, b1, W2, b2, W3, b3, _trace=False):
    global LAST_RESULTS
    shards, node_core, node_col, t_blk = _preprocess(x, edge_index, edge_attr)

    W1 = np.ascontiguousarray(np.asarray(W1, np.float32).astype(ml_dtypes.bfloat16))
    W2 = np.ascontiguousarray(np.asarray(W2, np.float32).astype(ml_dtypes.bfloat16))
    W3 = np.ascontiguousarray(np.asarray(W3, np.float32).astype(ml_dtypes.bfloat16))
    bb = np.concatenate(
        [
            np.asarray(b1, np.float32).reshape(4, 128).T,
            np.asarray(b2, np.float32).reshape(4, 128).T,
            np.asarray(b3, np.float32).reshape(1, 128).T,
        ],
        axis=1,
    )
    bb = np.ascontiguousarray(bb)

    in_maps = []
    for c in range(C):
        m = dict(shards[c])
        m.update(w1=W1, w2=W2, w3=W3, bb=bb)
        in_maps.append(m)

    nc = _build_program(t_blk)
    res = run_bass_kernel_spmd(nc, in_maps, core_ids=list(range(C)), trace=_trace)
    LAST_RESULTS = res

    outs = np.stack([res.results[c]["outT"] for c in range(C)])  # [C, 128, SLOTS] bf16
    out = outs.transpose(0, 2, 1)[node_core, node_col]
    return np.ascontiguousarray(out, dtype=np.float32)


# revision 46
# speedup vs baseline: 1.1786x; 1.1783x over previous
"""GNN message-passing (segment-mean + 3-layer MLP) Trainium2 kernel.

Strategy (8 NeuronCores, SPMD, full inputs in / full output out):
  - Host: assign nodes to 800 blocks of 64 slots (degree-balanced snake) so
    every block's incoming-edge count fits 6 k-tiles of 128 edges.  Blocks
    0-99 -> core 0, etc.  Edges are bucketed per receiver block, pre-scaled
    by 1/deg(recv), cast to bf16, and laid out [eslot, ktile*feat] so
    per-chunk DMAs are large and contiguous.  One-hot scatter masks are 64
    columns wide (fp8), halving mask DMA vs 128-wide blocks.
  - Device per core: segment-mean as mask matmuls on the TensorEngine (6
    k-tiles per block accumulated into one 512-col PSUM bank per chunk),
    then the 3-layer MLP over 512/256-node chunks in feature-major layout.
    Everything except PSUM/bias/mask is bf16: halves DMA bytes and enables
    fast-weight-load on the PE (f32r disables FWL).  ~3us of dummy matmuls
    at program start ramp the PE p-state while the DMA pipeline fills.
    Edge/mask slabs stream on the SP HWDGE ring in 2-4 block granules, 4
    chunks deep (the first two chunks are half-sized so compute starts
    early); x/weights/outputs ride the ACT ring, with the three biases
    fused into one transfer and x batched into 4-chunk spans.  Output is
    written bf16 and upcast on the host.
"""
import sys

sys.path.insert(0, "/opt/trn_rl_repo")

import numpy as np
import ml_dtypes

from concourse import bacc
import concourse.mybir as mybir
import concourse.tile as tile
from concourse.bass_utils import run_bass_kernel_spmd

# problem shape (hardcoded per contract)
N_NODES = 50000
N_EDGES = 600000
D = 128          # node/edge feature dim
DH = 512         # hidden dim
C = 8            # cores
W = 64           # node slots per block
BPC = 100        # node blocks per core
NB = C * BPC     # 800 blocks total
SLOTS = BPC * W  # 6400 node slots per core
T_BLK = 6        # edge k-tiles (128 edges) per block
TT = BPC * T_BLK   # k-tiles per core
CHUNKS = [4] * 4 + [8] * 10 + [4]  # blocks per MLP chunk (256-node ramp start)
PREF = 4         # chunks of edge-slab prefetch depth

F32 = mybir.dt.float32
BF16 = mybir.dt.bfloat16
FP8 = mybir.dt.float8e4

_prog_cache = {}
LAST_RESULTS = None  # BassKernelResults of the most recent run (for test.py)


def _build_program(t_blk=T_BLK):
    if t_blk in _prog_cache:
        return _prog_cache[t_blk]
    tt = BPC * t_blk
    nc = bacc.Bacc("TRN2", target_bir_lowering=False)

    xT_d = nc.declare_dram_parameter("xT", [128, SLOTS], BF16, isOutput=False)
    ea_d = nc.declare_dram_parameter("ea", [128, tt * D], BF16, isOutput=False)
    mk_d = nc.declare_dram_parameter("mk", [128, tt * W], FP8, isOutput=False)
    w1_d = nc.declare_dram_parameter("w1", [2 * D, DH], BF16, isOutput=False)
    w2_d = nc.declare_dram_parameter("w2", [DH, DH], BF16, isOutput=False)
    w3_d = nc.declare_dram_parameter("w3", [DH, D], BF16, isOutput=False)
    bb_d = nc.declare_dram_parameter("bb", [128, 9], F32, isOutput=False)
    out_d = nc.declare_dram_parameter("outT", [128, SLOTS], BF16, isOutput=True)

    RELU = mybir.ActivationFunctionType.Relu
    ADD = mybir.AluOpType.add

    n_chunks = len(CHUNKS)
    chunk_blk0 = [0]
    for c_i in range(1, n_chunks):
        chunk_blk0.append(chunk_blk0[-1] + CHUNKS[c_i - 1])

    with tile.TileContext(nc) as tc:
        with (
            tc.tile_pool(name="pers", bufs=1) as pers,
            tc.tile_pool(name="eap", bufs=12) as eap,
            tc.tile_pool(name="mkp", bufs=12) as mkp,
            tc.tile_pool(name="xp", bufs=4) as xp,
            tc.tile_pool(name="actp", bufs=2) as actp,
            tc.tile_pool(name="scat_ps", bufs=3, space="PSUM") as scat_ps,
            tc.tile_pool(name="mlp_ps", bufs=5, space="PSUM") as mlp_ps,
        ):
            # granules[(chunk, block)] = (ea_tile, mk_tile, local_block)
            gran = {}

            def load_granule(c_i, b0, nblk):
                kt0 = (chunk_blk0[c_i] + b0) * t_blk
                nkt = nblk * t_blk
                ea_t = eap.tile([128, 4 * t_blk * D], BF16, tag="ea")
                nc.sync.dma_start(
                    out=ea_t[:, : nkt * D], in_=ea_d[:, kt0 * D : (kt0 + nkt) * D]
                )
                mk_t = mkp.tile([128, 4 * t_blk * W], FP8, tag="mk")
                nc.sync.dma_start(
                    out=mk_t[:, : nkt * W], in_=mk_d[:, kt0 * W : (kt0 + nkt) * W]
                )
                for lb in range(nblk):
                    gran[(c_i, b0 + lb)] = (ea_t, mk_t, lb)

            def load_chunk_slabs(c_i, granule_blks):
                nb = CHUNKS[c_i]
                if isinstance(granule_blks, int):
                    granule_blks = [granule_blks] * ((nb + granule_blks - 1) // granule_blks)
                b0 = 0
                for g in granule_blks:
                    g = min(g, nb - b0)
                    if g <= 0:
                        break
                    load_granule(c_i, b0, g)
                    b0 += g

            x_ts = {}

            def load_x_span(c_lo, c_hi):
                col_lo = chunk_blk0[c_lo] * W
                col_hi = chunk_blk0[c_hi] * W + CHUNKS[c_hi] * W
                xt = xp.tile([128, 2816], BF16, tag="x")
                nc.scalar.dma_start(out=xt[:, : col_hi - col_lo], in_=xT_d[:, col_lo:col_hi])
                for c in range(c_lo, c_hi + 1):
                    x_ts[c] = (xt, chunk_blk0[c] * W - col_lo)

            # --- PE p-state warmup: ~3us of dummy matmuls with no deps run
            # during the DMA pipeline fill, so the real scatter/MLP start at
            # the full 2.4 GHz p-state instead of ramping through it ---
            warm_in = pers.tile([128, 128], BF16)
            nc.vector.memset(warm_in[:], 1.0)
            warm_rhs = pers.tile([128, 512], BF16)
            nc.vector.memset(warm_rhs[:], 1.0)
            warm_ps = scat_ps.tile([128, 512], F32, tag="scat")
            for wi in range(7):
                nc.tensor.matmul(
                    out=warm_ps[:],
                    lhsT=warm_in[:],
                    rhs=warm_rhs[:],
                    start=(wi == 0),
                    stop=(wi == 6),
                )

            # --- edge/mask slabs stream on the SP ring; chunk sizes ramp up
            # so the first scatter/MLP work starts ASAP ---
            load_chunk_slabs(0, 2)
            # --- persistent tiles + x on the ACT ring (w1 first: it gates
            # the first L1; fused biases ride one transfer) ---
            load_x_span(0, 0)
            w1t = pers.tile([128, 2, DH], BF16)
            nc.scalar.dma_start(out=w1t[:], in_=w1_d[:].rearrange("(k p) m -> p k m", p=128))
            bbt = pers.tile([128, 9], F32)
            nc.scalar.dma_start(out=bbt[:], in_=bb_d[:])
            load_chunk_slabs(1, 2)
            w2t = pers.tile([128, 4, DH], BF16)
            nc.scalar.dma_start(out=w2t[:], in_=w2_d[:].rearrange("(k p) m -> p k m", p=128))
            load_chunk_slabs(2, 4)
            w3t = pers.tile([128, 4, D], BF16)
            nc.scalar.dma_start(out=w3t[:], in_=w3_d[:].rearrange("(k p) m -> p k m", p=128))
            load_chunk_slabs(3, 4)
            load_x_span(1, 4)

            def emit_scatter(c_i, b_lo, b_hi, ps):
                for b in range(b_lo, b_hi):
                    ea_t, mk_t, lb = gran.pop((c_i, b))
                    for t in range(t_blk):
                        j = (lb * t_blk + t)
                        nc.tensor.matmul(
                            out=ps[:, b * W : (b + 1) * W],
                            lhsT=ea_t[:, j * D : (j + 1) * D],
                            rhs=mk_t[:, j * W : (j + 1) * W],
                            start=(t == 0),
                            stop=(t == t_blk - 1),
                        )

            col0 = 0
            ps_next = None
            pre_done = 0
            for c_i, nb in enumerate(CHUNKS):
                NCW = nb * W
                # scatter: segment-mean via mask matmuls, one PSUM bank per
                # chunk (a 2-block prefix may already have run, interleaved
                # after the previous chunk's L1)
                if ps_next is not None:
                    ps = ps_next
                else:
                    ps = scat_ps.tile([128, 512], F32, tag="scat")
                emit_scatter(c_i, pre_done, nb, ps)
                ps_next = None
                pre_done = 0
                mean_t = actp.tile([128, 512], BF16, tag="mean")
                nc.vector.tensor_copy(out=mean_t[:, :NCW], in_=ps[:, :NCW])

                # prefetch a later chunk's slabs / x
                if c_i + PREF < n_chunks:
                    load_chunk_slabs(c_i + PREF, 4)
                if c_i == 0:
                    load_x_span(5, 8)
                elif c_i == 2:
                    load_x_span(9, n_chunks - 1)

                # layer 1: h1 = relu(W1.T @ [x; mean] + b1).  All four
                # x-half matmuls issue first (they depend only on x), hiding
                # the DVE psum->SBUF mean-copy latency behind ~0.9us of PE work.
                h1_t = actp.tile([128, 4, 512], BF16, tag="h1")
                xt_c, xoff = x_ts[c_i]
                pms = []
                for m in range(4):
                    pm = mlp_ps.tile([128, 512], F32, tag="mlp")
                    nc.tensor.matmul(
                        out=pm[:, :NCW],
                        lhsT=w1t[:, 0, m * 128 : (m + 1) * 128],
                        rhs=xt_c[:, xoff : xoff + NCW],
                        start=True,
                        stop=False,
                    )
                    pms.append(pm)
                for m in range(4):
                    nc.tensor.matmul(
                        out=pms[m][:, :NCW],
                        lhsT=w1t[:, 1, m * 128 : (m + 1) * 128],
                        rhs=mean_t[:, :NCW],
                        start=False,
                        stop=True,
                    )
                    if m % 2 == 0:
                        nc.scalar.activation(
                            out=h1_t[:, m, :NCW], in_=pms[m][:, :NCW], func=RELU, bias=bbt[:, m : m + 1]
                        )
                    else:
                        nc.vector.tensor_scalar(
                            out=h1_t[:, m, :NCW], in0=pms[m][:, :NCW], scalar1=bbt[:, m : m + 1],
                            scalar2=0.0, op0=ADD, op1=mybir.AluOpType.max,
                        )
                # next chunk's scatter prefix: independent PE work that
                # bridges the L1-relu -> L2 dependency latency
                if c_i >= 4 and c_i + 1 < n_chunks:
                    ps_next = scat_ps.tile([128, 512], F32, tag="scat")
                    pre_done = min(2, CHUNKS[c_i + 1])
                    emit_scatter(c_i + 1, 0, pre_done, ps_next)

                # layer 2
                h2_t = actp.tile([128, 4, 512], BF16, tag="h2")
                for m in range(4):
                    pm = mlp_ps.tile([128, 512], F32, tag="mlp")
                    for k in range(4):
                        nc.tensor.matmul(
                            out=pm[:, :NCW],
                            lhsT=w2t[:, k, m * 128 : (m + 1) * 128],
                            rhs=h1_t[:, k, :NCW],
                            start=(k == 0),
                            stop=(k == 3),
                        )
                    if m % 2 == 0:
                        nc.scalar.activation(
                            out=h2_t[:, m, :NCW], in_=pm[:, :NCW], func=RELU, bias=bbt[:, 4 + m : 5 + m]
                        )
                    else:
                        nc.vector.tensor_scalar(
                            out=h2_t[:, m, :NCW], in0=pm[:, :NCW], scalar1=bbt[:, 4 + m : 5 + m],
                            scalar2=0.0, op0=ADD, op1=mybir.AluOpType.max,
                        )
                # layer 3: out = W3.T @ h2 + b3  (bias add on DVE, bf16 out)
                pm = mlp_ps.tile([128, 512], F32, tag="mlp")
                for k in range(4):
                    nc.tensor.matmul(
                        out=pm[:, :NCW],
                        lhsT=w3t[:, k, :],
                        rhs=h2_t[:, k, :NCW],
                        start=(k == 0),
                        stop=(k == 3),
                    )
                out_t = actp.tile([128, 512], BF16, tag="out")
                nc.vector.tensor_scalar_add(out_t[:, :NCW], pm[:, :NCW], bbt[:, 8:9])
                nc.scalar.dma_start(out=out_d[:, col0 : col0 + NCW], in_=out_t[:, :NCW])
                col0 += NCW

    nc.compile()
    _prog_cache[t_blk] = nc
    return nc


def _preprocess(x, edge_index, edge_attr):
    recv = np.asarray(edge_index)[1].astype(np.int64)
    deg = np.bincount(recv, minlength=N_NODES)
    # snake assignment of degree-sorted nodes into NB blocks (62-63 nodes each)
    order = np.argsort(-deg, kind="stable")
    i = np.arange(N_NODES)
    rnd, pos = i // NB, i % NB
    blk = np.where(rnd % 2 == 0, pos, NB - 1 - pos)
    node_block = np.empty(N_NODES, np.int64)
    node_slot = np.empty(N_NODES, np.int64)
    node_block[order] = blk
    node_slot[order] = rnd
    node_core = node_block // BPC
    node_col = (node_block % BPC) * W + node_slot

    eb = node_block[recv]
    bc = np.bincount(eb, minlength=NB)
    t_blk = max(T_BLK, int(-(-int(bc.max()) // 128)))  # >= ceil(max_load/128)
    tt = BPC * t_blk

    eorder = np.argsort(eb, kind="stable")
    eb_s = eb[eorder]
    starts = np.zeros(NB, np.int64)
    starts[1:] = np.cumsum(bc)[:-1]
    ewithin = np.arange(N_EDGES) - starts[eb_s]
    ktile = ewithin // 128
    eslot = ewithin % 128
    ecore = eb_s // BPC
    kt_in_core = (eb_s % BPC) * t_blk + ktile

    # scale edges by 1/deg(recv) on the host, then cast once to bf16
    ea_scaled = np.asarray(edge_attr, np.float32) * (1.0 / deg[recv])[:, None].astype(np.float32)
    ea_bf = ea_scaled.astype(ml_dtypes.bfloat16)
    ea_buf = np.zeros((C, tt, 128, D), ml_dtypes.bfloat16)
    ea_buf[ecore, kt_in_core, eslot] = ea_bf[eorder]
    # 0/1 scatter masks in fp8, 64 columns per block
    mk_buf = np.zeros((C, tt, 128, W), ml_dtypes.float8_e4m3)
    mk_buf[ecore, kt_in_core, eslot, (node_col[recv] % W)[eorder]] = 1.0

    X_all = np.zeros((C, SLOTS, D), ml_dtypes.bfloat16)
    X_all[node_core, node_col] = np.asarray(x, np.float32).astype(ml_dtypes.bfloat16)

    shards = []
    for c in range(C):
        shards.append(
            dict(
                xT=np.ascontiguousarray(X_all[c].T),
                ea=np.ascontiguousarray(ea_buf[c].transpose(1, 0, 2).reshape(128, tt * D)),
                mk=np.ascontiguousarray(mk_buf[c].transpose(1, 0, 2).reshape(128, tt * W)),
            )
        )
    return shards, node_core, node_col, t_blk


def kernel(x, edge_index, edge_attr, W1, b1, W2, b2, W3, b3, _trace=False):
    global LAST_RESULTS
    shards, node_core, node_col, t_blk = _preprocess(x, edge_index, edge_attr)

    W1 = np.ascontiguousarray(np.asarray(W1, np.float32).astype(ml_dtypes.bfloat16))
    W2 = np.ascontiguousarray(np.asarray(W2, np.float32).astype(ml_dtypes.bfloat16))
    W3 = np.ascontiguousarray(np.asarray(W3, np.float32).astype(ml_dtypes.bfloat16))
    bb = np.concatenate(
        [
            np.asarray(b1, np.float32).reshape(4, 128).T,
            np.asarray(b2, np.float32).reshape(4, 128).T,
            np.asarray(b3, np.float32).reshape(1, 128).T,
        ],
        axis=1,
    )
    bb = np.ascontiguousarray(bb)

    in_maps = []
    for c in range(C):
        m = dict(shards[c])
        m.update(w1=W1, w2=W2, w3=W3, bb=bb)
        in_maps.append(m)

    nc = _build_program(t_blk)
    res = run_bass_kernel_spmd(nc, in_maps, core_ids=list(range(C)), trace=_trace)
    LAST_RESULTS = res

    outs = np.stack([res.results[c]["outT"] for c in range(C)])  # [C, 128, SLOTS] bf16
    out = outs.transpose(0, 2, 1)[node_core, node_col]
    return np.ascontiguousarray(out, dtype=np.float32)


# revision 47
# speedup vs baseline: 1.1821x; 1.0029x over previous
"""GNN message-passing (segment-mean + 3-layer MLP) Trainium2 kernel.

Strategy (8 NeuronCores, SPMD, full inputs in / full output out):
  - Host: assign nodes to 800 blocks of 64 slots (degree-balanced snake) so
    every block's incoming-edge count fits 6 k-tiles of 128 edges.  Blocks
    0-99 -> core 0, etc.  Edges are bucketed per receiver block, pre-scaled
    by 1/deg(recv), cast to bf16, and laid out [eslot, ktile*feat] so
    per-chunk DMAs are large and contiguous.  One-hot scatter masks are 64
    columns wide (fp8), halving mask DMA vs 128-wide blocks.
  - Device per core: segment-mean as mask matmuls on the TensorEngine (6
    k-tiles per block accumulated into one 512-col PSUM bank per chunk),
    then the 3-layer MLP over 512/256-node chunks in feature-major layout.
    Everything except PSUM/bias/mask is bf16: halves DMA bytes and enables
    fast-weight-load on the PE (f32r disables FWL).  ~3us of dummy matmuls
    at program start ramp the PE p-state while the DMA pipeline fills.
    Edge/mask slabs stream on the SP HWDGE ring in 2-4 block granules, 4
    chunks deep (the first two chunks are half-sized so compute starts
    early); x/weights/outputs ride the ACT ring, with the three biases
    fused into one transfer and x batched into 4-chunk spans.  Output is
    written bf16 and upcast on the host.
"""
import sys

sys.path.insert(0, "/opt/trn_rl_repo")

import numpy as np
import ml_dtypes

from concourse import bacc
import concourse.mybir as mybir
import concourse.tile as tile
from concourse.bass_utils import run_bass_kernel_spmd

# problem shape (hardcoded per contract)
N_NODES = 50000
N_EDGES = 600000
D = 128          # node/edge feature dim
DH = 512         # hidden dim
C = 8            # cores
W = 64           # node slots per block
BPC = 100        # node blocks per core
NB = C * BPC     # 800 blocks total
SLOTS = BPC * W  # 6400 node slots per core
T_BLK = 6        # edge k-tiles (128 edges) per block
TT = BPC * T_BLK   # k-tiles per core
CHUNKS = [4] * 4 + [8] * 10 + [4]  # blocks per MLP chunk (256-node ramp start)
PREF = 4         # chunks of edge-slab prefetch depth

F32 = mybir.dt.float32
BF16 = mybir.dt.bfloat16
FP8 = mybir.dt.float8e4

_prog_cache = {}
LAST_RESULTS = None  # BassKernelResults of the most recent run (for test.py)


def _build_program(t_blk=T_BLK):
    if t_blk in _prog_cache:
        return _prog_cache[t_blk]
    tt = BPC * t_blk
    nc = bacc.Bacc("TRN2", target_bir_lowering=False)

    xT_d = nc.declare_dram_parameter("xT", [128, SLOTS], BF16, isOutput=False)
    ea_d = nc.declare_dram_parameter("ea", [128, tt * D], BF16, isOutput=False)
    mk_d = nc.declare_dram_parameter("mk", [128, tt * W], FP8, isOutput=False)
    w1_d = nc.declare_dram_parameter("w1", [2 * D, DH], BF16, isOutput=False)
    w2_d = nc.declare_dram_parameter("w2", [DH, DH], BF16, isOutput=False)
    w3_d = nc.declare_dram_parameter("w3", [DH, D], BF16, isOutput=False)
    bb_d = nc.declare_dram_parameter("bb", [128, 9], F32, isOutput=False)
    out_d = nc.declare_dram_parameter("outT", [128, SLOTS], BF16, isOutput=True)

    RELU = mybir.ActivationFunctionType.Relu
    ADD = mybir.AluOpType.add

    n_chunks = len(CHUNKS)
    chunk_blk0 = [0]
    for c_i in range(1, n_chunks):
        chunk_blk0.append(chunk_blk0[-1] + CHUNKS[c_i - 1])

    with tile.TileContext(nc) as tc:
        with (
            tc.tile_pool(name="pers", bufs=1) as pers,
            tc.tile_pool(name="eap", bufs=12) as eap,
            tc.tile_pool(name="mkp", bufs=12) as mkp,
            tc.tile_pool(name="xp", bufs=4) as xp,
            tc.tile_pool(name="actp", bufs=2) as actp,
            tc.tile_pool(name="scat_ps", bufs=3, space="PSUM") as scat_ps,
            tc.tile_pool(name="mlp_ps", bufs=5, space="PSUM") as mlp_ps,
        ):
            # granules[(chunk, block)] = (ea_tile, mk_tile, local_block)
            gran = {}

            def load_granule(c_i, b0, nblk):
                kt0 = (chunk_blk0[c_i] + b0) * t_blk
                nkt = nblk * t_blk
                ea_t = eap.tile([128, 4 * t_blk * D], BF16, tag="ea")
                nc.sync.dma_start(
                    out=ea_t[:, : nkt * D], in_=ea_d[:, kt0 * D : (kt0 + nkt) * D]
                )
                mk_t = mkp.tile([128, 4 * t_blk * W], FP8, tag="mk")
                nc.sync.dma_start(
                    out=mk_t[:, : nkt * W], in_=mk_d[:, kt0 * W : (kt0 + nkt) * W]
                )
                for lb in range(nblk):
                    gran[(c_i, b0 + lb)] = (ea_t, mk_t, lb)

            def load_chunk_slabs(c_i, granule_blks):
                nb = CHUNKS[c_i]
                if isinstance(granule_blks, int):
                    granule_blks = [granule_blks] * ((nb + granule_blks - 1) // granule_blks)
                b0 = 0
                for g in granule_blks:
                    g = min(g, nb - b0)
                    if g <= 0:
                        break
                    load_granule(c_i, b0, g)
                    b0 += g

            x_ts = {}

            def load_x_span(c_lo, c_hi):
                col_lo = chunk_blk0[c_lo] * W
                col_hi = chunk_blk0[c_hi] * W + CHUNKS[c_hi] * W
                xt = xp.tile([128, 2816], BF16, tag="x")
                nc.scalar.dma_start(out=xt[:, : col_hi - col_lo], in_=xT_d[:, col_lo:col_hi])
                for c in range(c_lo, c_hi + 1):
                    x_ts[c] = (xt, chunk_blk0[c] * W - col_lo)

            # --- PE p-state warmup: ~3us of dummy matmuls with no deps run
            # during the DMA pipeline fill, so the real scatter/MLP start at
            # the full 2.4 GHz p-state instead of ramping through it ---
            warm_in = pers.tile([128, 128], BF16)
            nc.vector.memset(warm_in[:], 1.0)
            warm_rhs = pers.tile([128, 512], BF16)
            nc.vector.memset(warm_rhs[:], 1.0)
            warm_ps = scat_ps.tile([128, 512], F32, tag="scat")
            for wi in range(7):
                nc.tensor.matmul(
                    out=warm_ps[:],
                    lhsT=warm_in[:],
                    rhs=warm_rhs[:],
                    start=(wi == 0),
                    stop=(wi == 6),
                )

            # --- edge/mask slabs stream on the SP ring; chunk sizes ramp up
            # so the first scatter/MLP work starts ASAP ---
            load_chunk_slabs(0, 2)
            # --- persistent tiles + x on the ACT ring (w1 first: it gates
            # the first L1; fused biases ride one transfer) ---
            load_x_span(0, 0)
            w1t = pers.tile([128, 2, DH], BF16)
            nc.scalar.dma_start(out=w1t[:], in_=w1_d[:].rearrange("(k p) m -> p k m", p=128))
            bbt = pers.tile([128, 9], F32)
            nc.scalar.dma_start(out=bbt[:], in_=bb_d[:])
            load_chunk_slabs(1, 2)
            w2t = pers.tile([128, 4, DH], BF16)
            nc.scalar.dma_start(out=w2t[:], in_=w2_d[:].rearrange("(k p) m -> p k m", p=128))
            load_chunk_slabs(2, 4)
            w3t = pers.tile([128, 4, D], BF16)
            nc.scalar.dma_start(out=w3t[:], in_=w3_d[:].rearrange("(k p) m -> p k m", p=128))
            load_chunk_slabs(3, 4)
            load_x_span(1, 4)

            def emit_scatter(c_i, b_lo, b_hi, ps):
                for b in range(b_lo, b_hi):
                    ea_t, mk_t, lb = gran.pop((c_i, b))
                    for t in range(t_blk):
                        j = (lb * t_blk + t)
                        nc.tensor.matmul(
                            out=ps[:, b * W : (b + 1) * W],
                            lhsT=ea_t[:, j * D : (j + 1) * D],
                            rhs=mk_t[:, j * W : (j + 1) * W],
                            start=(t == 0),
                            stop=(t == t_blk - 1),
                        )

            col0 = 0
            ps_next = None
            pre_done = 0
            for c_i, nb in enumerate(CHUNKS):
                NCW = nb * W
                # scatter: segment-mean via mask matmuls, one PSUM bank per
                # chunk (a 2-block prefix may already have run, interleaved
                # after the previous chunk's L1)
                if ps_next is not None:
                    ps = ps_next
                else:
                    ps = scat_ps.tile([128, 512], F32, tag="scat")
                emit_scatter(c_i, pre_done, nb, ps)
                ps_next = None
                pre_done = 0
                mean_t = actp.tile([128, 512], BF16, tag="mean")
                nc.vector.tensor_copy(out=mean_t[:, :NCW], in_=ps[:, :NCW])

                # prefetch a later chunk's slabs / x
                if c_i + PREF < n_chunks:
                    load_chunk_slabs(c_i + PREF, 4)
                if c_i == 3:
                    load_x_span(5, 8)
                elif c_i == 6:
                    load_x_span(9, n_chunks - 1)

                # layer 1: h1 = relu(W1.T @ [x; mean] + b1).  All four
                # x-half matmuls issue first (they depend only on x), hiding
                # the DVE psum->SBUF mean-copy latency behind ~0.9us of PE work.
                h1_t = actp.tile([128, 4, 512], BF16, tag="h1")
                xt_c, xoff = x_ts[c_i]
                pms = []
                for m in range(4):
                    pm = mlp_ps.tile([128, 512], F32, tag="mlp")
                    nc.tensor.matmul(
                        out=pm[:, :NCW],
                        lhsT=w1t[:, 0, m * 128 : (m + 1) * 128],
                        rhs=xt_c[:, xoff : xoff + NCW],
                        start=True,
                        stop=False,
                    )
                    pms.append(pm)
                for m in range(4):
                    nc.tensor.matmul(
                        out=pms[m][:, :NCW],
                        lhsT=w1t[:, 1, m * 128 : (m + 1) * 128],
                        rhs=mean_t[:, :NCW],
                        start=False,
                        stop=True,
                    )
                    if m % 2 == 0:
                        nc.scalar.activation(
                            out=h1_t[:, m, :NCW], in_=pms[m][:, :NCW], func=RELU, bias=bbt[:, m : m + 1]
                        )
                    else:
                        nc.vector.tensor_scalar(
                            out=h1_t[:, m, :NCW], in0=pms[m][:, :NCW], scalar1=bbt[:, m : m + 1],
                            scalar2=0.0, op0=ADD, op1=mybir.AluOpType.max,
                        )
                # next chunk's scatter prefix: independent PE work that
                # bridges the L1-relu -> L2 dependency latency
                if c_i >= 4 and c_i + 1 < n_chunks:
                    ps_next = scat_ps.tile([128, 512], F32, tag="scat")
                    pre_done = min(2, CHUNKS[c_i + 1])
                    emit_scatter(c_i + 1, 0, pre_done, ps_next)

                # layer 2
                h2_t = actp.tile([128, 4, 512], BF16, tag="h2")
                for m in range(4):
                    pm = mlp_ps.tile([128, 512], F32, tag="mlp")
                    for k in range(4):
                        nc.tensor.matmul(
                            out=pm[:, :NCW],
                            lhsT=w2t[:, k, m * 128 : (m + 1) * 128],
                            rhs=h1_t[:, k, :NCW],
                            start=(k == 0),
                            stop=(k == 3),
                        )
                    if m % 2 == 0:
                        nc.scalar.activation(
                            out=h2_t[:, m, :NCW], in_=pm[:, :NCW], func=RELU, bias=bbt[:, 4 + m : 5 + m]
                        )
                    else:
                        nc.vector.tensor_scalar(
                            out=h2_t[:, m, :NCW], in0=pm[:, :NCW], scalar1=bbt[:, 4 + m : 5 + m],
                            scalar2=0.0, op0=ADD, op1=mybir.AluOpType.max,
                        )
                # layer 3: out = W3.T @ h2 + b3  (bias add on DVE, bf16 out)
                pm = mlp_ps.tile([128, 512], F32, tag="mlp")
                for k in range(4):
                    nc.tensor.matmul(
                        out=pm[:, :NCW],
                        lhsT=w3t[:, k, :],
                        rhs=h2_t[:, k, :NCW],
                        start=(k == 0),
                        stop=(k == 3),
                    )
                out_t = actp.tile([128, 512], BF16, tag="out")
                nc.vector.tensor_scalar_add(out_t[:, :NCW], pm[:, :NCW], bbt[:, 8:9])
                nc.scalar.dma_start(out=out_d[:, col0 : col0 + NCW], in_=out_t[:, :NCW])
                col0 += NCW

    nc.compile()
    _prog_cache[t_blk] = nc
    return nc


def _preprocess(x, edge_index, edge_attr):
    recv = np.asarray(edge_index)[1].astype(np.int64)
    deg = np.bincount(recv, minlength=N_NODES)
    # snake assignment of degree-sorted nodes into NB blocks (62-63 nodes each)
    order = np.argsort(-deg, kind="stable")
    i = np.arange(N_NODES)
    rnd, pos = i // NB, i % NB
    blk = np.where(rnd % 2 == 0, pos, NB - 1 - pos)
    node_block = np.empty(N_NODES, np.int64)
    node_slot = np.empty(N_NODES, np.int64)
    node_block[order] = blk
    node_slot[order] = rnd
    node_core = node_block // BPC
    node_col = (node_block % BPC) * W + node_slot

    eb = node_block[recv]
    bc = np.bincount(eb, minlength=NB)
    t_blk = max(T_BLK, int(-(-int(bc.max()) // 128)))  # >= ceil(max_load/128)
    tt = BPC * t_blk

    eorder = np.argsort(eb, kind="stable")
    eb_s = eb[eorder]
    starts = np.zeros(NB, np.int64)
    starts[1:] = np.cumsum(bc)[:-1]
    ewithin = np.arange(N_EDGES) - starts[eb_s]
    ktile = ewithin // 128
    eslot = ewithin % 128
    ecore = eb_s // BPC
    kt_in_core = (eb_s % BPC) * t_blk + ktile

    # scale edges by 1/deg(recv) on the host, then cast once to bf16
    ea_scaled = np.asarray(edge_attr, np.float32) * (1.0 / deg[recv])[:, None].astype(np.float32)
    ea_bf = ea_scaled.astype(ml_dtypes.bfloat16)
    ea_buf = np.zeros((C, tt, 128, D), ml_dtypes.bfloat16)
    ea_buf[ecore, kt_in_core, eslot] = ea_bf[eorder]
    # 0/1 scatter masks in fp8, 64 columns per block
    mk_buf = np.zeros((C, tt, 128, W), ml_dtypes.float8_e4m3)
    mk_buf[ecore, kt_in_core, eslot, (node_col[recv] % W)[eorder]] = 1.0

    X_all = np.zeros((C, SLOTS, D), ml_dtypes.bfloat16)
    X_all[node_core, node_col] = np.asarray(x, np.float32).astype(ml_dtypes.bfloat16)

    shards = []
    for c in range(C):
        shards.append(
            dict(
                xT=np.ascontiguousarray(X_all[c].T),
                ea=np.ascontiguousarray(ea_buf[c].transpose(1, 0, 2).reshape(128, tt * D)),
                mk=np.ascontiguousarray(mk_buf[c].transpose(1, 0, 2).reshape(128, tt * W)),
            )
        )
    return shards, node_core, node_col, t_blk


def kernel(x, edge_index, edge_attr, W1, b1, W2, b2, W3, b3, _trace=False):
    global LAST_RESULTS
    shards, node_core, node_col, t_blk = _preprocess(x, edge_index, edge_attr)

    W1 = np.ascontiguousarray(np.asarray(W1, np.float32).astype(ml_dtypes.bfloat16))
    W2 = np.ascontiguousarray(np.asarray(W2, np.float32).astype(ml_dtypes.bfloat16))
    W3 = np.ascontiguousarray(np.asarray(W3, np.float32).astype(ml_dtypes.bfloat16))
    bb = np.concatenate(
        [
            np.asarray(b1, np.float32).reshape(4, 128).T,
            np.asarray(b2, np.float32).reshape(4, 128).T,
            np.asarray(b3, np.float32).reshape(1, 128).T,
        ],
        axis=1,
    )
    bb = np.ascontiguousarray(bb)

    in_maps = []
    for c in range(C):
        m = dict(shards[c])
        m.update(w1=W1, w2=W2, w3=W3, bb=bb)
        in_maps.append(m)

    nc = _build_program(t_blk)
    res = run_bass_kernel_spmd(nc, in_maps, core_ids=list(range(C)), trace=_trace)
    LAST_RESULTS = res

    outs = np.stack([res.results[c]["outT"] for c in range(C)])  # [C, 128, SLOTS] bf16
    out = outs.transpose(0, 2, 1)[node_core, node_col]
    return np.ascontiguousarray(out, dtype=np.float32)
